# revision 9
# baseline (speedup 1.0000x reference)
"""Trainium2 Bass kernel for nn_AttentionBlock (B=8, L=2048, C=512, GroupNorm(8) +
single-head attention + residual), data-parallel over batch across 8 NeuronCores.

Self-contained: hardcodes shapes/sharding. kernel(**inputs) -> np.ndarray [B,L,C].

Dataflow (per core, one batch element, everything channel-major / "transposed"):
  x^T [C,L] (f32) --bn_stats/group-reduce--> h^T = a_c * x^T + b_c  (f32 + bf16 copy)
  Q^T = wq^T h^T + bq ;  K^T = (wk*scale)^T h^T + bk*scale  (scale folded on host)
  V   = h^T-chunks^T @ wv + bv           (natural [L,C] layout)
  per 512-wide lq tile:
     for each 128-key block: S^T = K^T-chunk^T @ Q^T (PSUM f32); P = exp(S^T) (bf16)
     O^T  += V-chunk^T @ P  (PSUM f32 accum over key blocks), denom += 1^T @ P
     out^T = h^T + (wp^T O^T) * (1/denom) + bp      (f32 combine)
Matmul operands are bf16 (1 cyc/row on PE); accumulation always fp32 in PSUM.
Host side transposes x per batch, casts weights to bf16, transposes output back.
"""

import numpy as np

B, L, C = 8, 2048, 512
GROUPS = 8
EPS = 1e-3
P = 128
CS = C // P            # 4 channel subtiles of 128
LQ = 512               # lq tile width (matmul free dim)
NLT = L // LQ          # 4 lq tiles
NLB = L // P           # 16 key/l blocks
CPG = C // GROUPS      # 64 channels per group
N_CORES = 8

_CACHE = {}


def _build_nc():
    from contextlib import ExitStack

    import concourse.bass as bass
    import concourse.mybir as mybir
    import concourse.tile as tile
    from concourse import bacc
    from concourse.bass import ts

    f32 = mybir.dt.float32
    bf16 = mybir.dt.bfloat16
    AF = mybir.ActivationFunctionType
    ALU = mybir.AluOpType

    nc = bacc.Bacc(trn_type="TRN2")

    xt_d = nc.dram_tensor("xt", [C, L], f32, kind="ExternalInput")
    w_d = {
        n: nc.dram_tensor(n, [P, CS, C], bf16, kind="ExternalInput")
        for n in ("wq", "wk", "wv", "wp")
    }
    vec_d = {
        n: nc.dram_tensor(n, [P, CS], f32, kind="ExternalInput")
        for n in ("gamma", "beta", "bq", "bk", "bp")
    }
    bvb_d = nc.dram_tensor("bv_bcast", [P, C], f32, kind="ExternalInput")
    g0_d = nc.dram_tensor("g0", [P, 2], f32, kind="ExternalInput")
    sel_d = nc.dram_tensor("sel", [2, P], f32, kind="ExternalInput")
    out_d = nc.dram_tensor("out_t", [C, L], f32, kind="ExternalOutput")

    xt_dv = xt_d[:].rearrange("(s p) l -> p s l", p=P)
    out_dv = out_d[:].rearrange("(s p) l -> p s l", p=P)

    with tile.TileContext(nc) as tc, ExitStack() as ctx:
        consts = ctx.enter_context(tc.tile_pool(name="consts", bufs=1))
        data = ctx.enter_context(tc.tile_pool(name="data", bufs=1))
        small = ctx.enter_context(tc.tile_pool(name="small", bufs=1))
        ptp = ctx.enter_context(tc.tile_pool(name="ptp", bufs=3))
        oup = ctx.enter_context(tc.tile_pool(name="oup", bufs=4))
        finp = ctx.enter_context(tc.tile_pool(name="finp", bufs=2))
        psA = ctx.enter_context(tc.tile_pool(name="psA", bufs=4, space="PSUM"))
        psS = ctx.enter_context(tc.tile_pool(name="psS", bufs=3, space="PSUM"))
        psD = ctx.enter_context(tc.tile_pool(name="psD", bufs=1, space="PSUM"))

        # ---- SBUF residents ----
        xt = data.tile([P, CS, L], f32)       # x^T, then h^T (f32, residual)
        hb = data.tile([P, CS, L], bf16)      # h^T bf16 (matmul operand)
        qt = data.tile([P, CS, L], bf16)      # Q^T
        kt = data.tile([P, CS, L], bf16)      # K^T (pre-scaled)
        vt = data.tile([P, NLB, C], bf16)     # V natural, [l%P, l//P, c]
        wsb = {n: consts.tile([P, CS, C], bf16, name=f"w_{n}") for n in w_d}
        vsb = {n: consts.tile([P, CS], f32, name=f"v_{n}") for n in vec_d}
        bvb = consts.tile([P, C], f32)
        g0 = consts.tile([P, 2], f32)
        sel = consts.tile([2, P], f32)
        ones_col = consts.tile([P, 1], bf16)
        ones_row = consts.tile([1, P], bf16)
        eps2 = consts.tile([2, 1], f32)

        # ---- loads + constants ----
        for s in range(CS):
            for j in range(4):
                nc.sync.dma_start(out=xt[:, s, ts(j, 512)], in_=xt_dv[:, s, ts(j, 512)])
        for n in w_d:
            nc.sync.dma_start(out=wsb[n][:], in_=w_d[n][:])
        for n in vec_d:
            nc.sync.dma_start(out=vsb[n][:], in_=vec_d[n][:])
        nc.sync.dma_start(out=bvb[:], in_=bvb_d[:])
        nc.sync.dma_start(out=g0[:], in_=g0_d[:])
        nc.sync.dma_start(out=sel[:], in_=sel_d[:])
        nc.vector.memset(ones_col[:], 1.0)
        nc.vector.memset(ones_row[:], 1.0)
        nc.vector.memset(eps2[:], EPS)

        # ---- GroupNorm stats ----
        # per-channel (partition) mean/var over L via bn_stats, then group
        # aggregation across partitions with a tiny fp32 matmul.
        st = small.tile([P, CS, 2], f32)      # (mean_c, E[x^2]_c) per subtile
        for s in range(CS):
            st6 = small.tile([P, 4, 6], f32, tag="st6", bufs=2)
            for j in range(4):
                nc.vector.bn_stats(out=st6[:, j, :], in_=xt[:, s, ts(j, 512)])
            mv = small.tile([P, 2], f32, tag="mv", bufs=2)
            nc.vector.bn_aggr(out=mv[:], in_=st6[:])
            nc.vector.tensor_copy(out=st[:, s, 0:1], in_=mv[:, 0:1])
            nc.vector.tensor_tensor(out=st[:, s, 1:2], in0=mv[:, 0:1], in1=mv[:, 0:1], op=ALU.mult)
            nc.vector.tensor_tensor(out=st[:, s, 1:2], in0=st[:, s, 1:2], in1=mv[:, 1:2], op=ALU.add)

        psg = psD.tile([2, 2 * CS], f32, tag="d")   # [group-half, (s, stat)]
        nc.tensor.matmul(psg[:], lhsT=g0[:], rhs=st[:].rearrange("p a b -> p (a b)"),
                         start=True, stop=True)
        pst = small.tile([2, 2 * CS], f32)
        nc.vector.tensor_copy(out=pst[:], in_=psg[:])
        pstv = pst[:].rearrange("p (s k) -> p s k", k=2)
        msq = small.tile([2, CS], f32)
        nc.vector.tensor_tensor(out=msq[:], in0=pstv[:, :, 0], in1=pstv[:, :, 0], op=ALU.mult)
        grp = small.tile([2, 2 * CS], f32)     # [:, :CS]=rstd_g, [:, CS:]=mean_g
        nc.vector.tensor_tensor(out=grp[:, 0:CS], in0=pstv[:, :, 1], in1=msq[:], op=ALU.subtract)
        nc.scalar.activation(out=grp[:, 0:CS], in_=grp[:, 0:CS], func=AF.Sqrt,
                             bias=eps2[:], scale=1.0)
        nc.vector.reciprocal(out=grp[:, 0:CS], in_=grp[:, 0:CS])
        nc.vector.tensor_copy(out=grp[:, CS:], in_=pstv[:, :, 0])

        psbc = psD.tile([P, 2 * CS], f32, tag="d")  # broadcast groups -> channels
        nc.tensor.matmul(psbc[:], lhsT=sel[:], rhs=grp[:], start=True, stop=True)
        ab = small.tile([P, 2 * CS], f32)      # [:, :CS]=a_c, [:, CS:]=b_c
        nc.vector.tensor_tensor(out=ab[:, 0:CS], in0=vsb["gamma"][:], in1=psbc[:, 0:CS], op=ALU.mult)
        nc.vector.tensor_tensor(out=ab[:, CS:], in0=psbc[:, CS:], in1=ab[:, 0:CS], op=ALU.mult)
        nc.vector.tensor_tensor(out=ab[:, CS:], in0=vsb["beta"][:], in1=ab[:, CS:], op=ALU.subtract)

        # ---- normalize: h^T = a*x^T + b (f32 in place, bf16 copy) ----
        for s in range(CS):
            nc.vector.tensor_scalar(out=xt[:, s, :], in0=xt[:, s, :],
                                    scalar1=ab[:, s:s + 1], scalar2=ab[:, CS + s:CS + s + 1],
                                    op0=ALU.mult, op1=ALU.add)
            nc.vector.tensor_copy(out=hb[:, s, :], in_=xt[:, s, :])

        # ---- projections ----
        def project_t(w, bias, dst):
            # dst[:, co_s, l] = sum_ci w[ci, co]^T h^T + bias[co]
            for co_s in range(CS):
                for lt in range(NLT):
                    ps = psS.tile([P, LQ], f32, tag="s", name="ps_prj")
                    for ci in range(CS):
                        nc.tensor.matmul(ps[:], lhsT=w[:, ci, ts(co_s, P)],
                                         rhs=hb[:, ci, ts(lt, LQ)],
                                         start=(ci == 0), stop=(ci == CS - 1))
                    nc.scalar.activation(out=dst[:, co_s, ts(lt, LQ)], in_=ps[:],
                                         func=AF.Identity, bias=bias[:, co_s:co_s + 1], scale=1.0)

        project_t(wsb["wq"], vsb["bq"], qt)
        project_t(wsb["wk"], vsb["bk"], kt)

        for lb in range(NLB):
            ps = psS.tile([P, C], f32, tag="s", name="ps_v")
            for ci in range(CS):
                nc.tensor.matmul(ps[:], lhsT=hb[:, ci, ts(lb, P)],
                                 rhs=wsb["wv"][:, ci, :],
                                 start=(ci == 0), stop=(ci == CS - 1))
            nc.vector.tensor_add(out=vt[:, lb, :], in0=ps[:], in1=bvb[:])

        # ---- attention + output projection, per lq tile ----
        for lt in range(NLT):
            po = [psA.tile([P, LQ], f32, tag="po", name=f"po{i}") for i in range(CS)]
            pd = psD.tile([1, LQ], f32, tag="d", name="pd")
            for kb in range(NLB):
                ps = psS.tile([P, LQ], f32, tag="s", name="ps_s")
                for ci in range(CS):
                    nc.tensor.matmul(ps[:], lhsT=kt[:, ci, ts(kb, P)],
                                     rhs=qt[:, ci, ts(lt, LQ)],
                                     start=(ci == 0), stop=(ci == CS - 1))
                pt = ptp.tile([P, LQ], bf16, tag="pt")
                nc.scalar.activation(out=pt[:], in_=ps[:], func=AF.Exp)
                for c_ in range(CS):
                    nc.tensor.matmul(po[c_][:], lhsT=vt[:, kb, ts(c_, P)], rhs=pt[:],
                                     start=(kb == 0), stop=(kb == NLB - 1))
                nc.tensor.matmul(pd[:], lhsT=ones_col[:], rhs=pt[:],
                                 start=(kb == 0), stop=(kb == NLB - 1))

            rd = small.tile([1, LQ], bf16, tag="rd", bufs=2)
            with nc.allow_low_precision(reason="1/denom rounded to bf16 as matmul operand"):
                nc.vector.reciprocal(out=rd[:], in_=pd[:])
            pb = psS.tile([P, LQ], f32, tag="s", name="ps_b")
            nc.tensor.matmul(pb[:], lhsT=ones_row[:], rhs=rd[:], start=True, stop=True)
            rb = finp.tile([P, LQ], f32, tag="rb")
            nc.vector.tensor_copy(out=rb[:], in_=pb[:])

            ou = [oup.tile([P, LQ], bf16, tag="ou", name=f"ou{i}") for i in range(CS)]
            for c_ in range(CS):
                nc.scalar.copy(out=ou[c_][:], in_=po[c_][:])

            for co_s in range(CS):
                pz = psS.tile([P, LQ], f32, tag="s", name="ps_z")
                for ci in range(CS):
                    nc.tensor.matmul(pz[:], lhsT=wsb["wp"][:, ci, ts(co_s, P)],
                                     rhs=ou[ci][:],
                                     start=(ci == 0), stop=(ci == CS - 1))
                fin = finp.tile([P, LQ], f32, tag="fin")
                nc.vector.tensor_tensor(out=fin[:], in0=pz[:], in1=rb[:], op=ALU.mult)
                nc.vector.tensor_scalar(out=fin[:], in0=fin[:],
                                        scalar1=vsb["bp"][:, co_s:co_s + 1], scalar2=None,
                                        op0=ALU.add)
                nc.vector.tensor_tensor(out=fin[:], in0=fin[:],
                                        in1=xt[:, co_s, ts(lt, LQ)], op=ALU.add)
                nc.sync.dma_start(out=out_dv[:, co_s, ts(lt, LQ)], in_=fin[:])

    nc.compile()
    return nc


def get_nc():
    if "nc" not in _CACHE:
        _CACHE["nc"] = _build_nc()
    return _CACHE["nc"]


def _g0_const():
    g = np.zeros((P, 2), np.float32)
    g[0:CPG, 0] = 1.0 / CPG
    g[CPG:P, 1] = 1.0 / CPG
    return g


def _sel_const():
    s = np.zeros((2, P), np.float32)
    s[0, 0:CPG] = 1.0
    s[1, CPG:P] = 1.0
    return s


def prep_inputs(x, gamma, beta, wq, bq, wk, bk, wv, bv, wp, bp):
    """Host-side layout prep (transposes / reshapes / bf16 weight casts, plus
    folding the 1/sqrt(C) attention scale into wk/bk). Per-core input maps."""
    import ml_dtypes

    f = np.float32
    bf = ml_dtypes.bfloat16
    x = np.asarray(x, f)
    scale = f(C) ** f(-0.5)

    def wprep(w):
        w = np.asarray(w, f)
        return np.ascontiguousarray(w.reshape(CS, P, C).transpose(1, 0, 2)).astype(bf)

    def vprep(v):
        v = np.asarray(v, f)
        return np.ascontiguousarray(v.reshape(CS, P).T)

    shared = {
        "wq": wprep(wq), "wk": wprep(np.asarray(wk, f) * scale),
        "wv": wprep(wv), "wp": wprep(wp),
        "gamma": vprep(gamma), "beta": vprep(beta),
        "bq": vprep(bq), "bk": vprep(np.asarray(bk, f) * scale), "bp": vprep(bp),
        "bv_bcast": np.ascontiguousarray(np.broadcast_to(np.asarray(bv, f), (P, C))),
        "g0": _g0_const(), "sel": _sel_const(),
    }
    in_maps = []
    for b in range(N_CORES):
        m = dict(shared)
        m["xt"] = np.ascontiguousarray(x[b].T)
        in_maps.append(m)
    return in_maps


def run(inputs, trace=False, **kw):
    from concourse.bass_utils import run_bass_kernel_spmd

    nc = get_nc()
    in_maps = prep_inputs(**inputs)
    return run_bass_kernel_spmd(nc, in_maps, core_ids=list(range(N_CORES)),
                                trace=trace, **kw)


def kernel(**inputs) -> np.ndarray:
    res = run(inputs)
    out = np.empty((B, L, C), np.float32)
    for b in range(N_CORES):
        out[b] = res.results[b]["out_t"].T
    return out


# revision 11
# speedup vs baseline: 1.0518x; 1.0518x over previous
"""Trainium2 Bass kernel for nn_AttentionBlock (B=8, L=2048, C=512, GroupNorm(8) +
single-head attention + residual), data-parallel over batch across 8 NeuronCores.

Self-contained: hardcodes shapes/sharding. kernel(**inputs) -> np.ndarray [B,L,C].

Dataflow (per core, one batch element, everything channel-major / "transposed"):
  x^T [C,L] (f32) --bn_stats/group-reduce--> h^T = a_c * x^T + b_c  (f32 + bf16 copy)
  Q^T = wq^T h^T + bq ;  K^T = (wk*scale)^T h^T + bk*scale  (scale folded on host)
  V   = h^T-chunks^T @ wv + bv           (natural [L,C] layout)
  per 512-wide lq tile:
     for each 128-key block: S^T = K^T-chunk^T @ Q^T (PSUM f32); P = exp(S^T) (bf16)
     O^T  += V-chunk^T @ P  (PSUM f32 accum over key blocks), denom += 1^T @ P
     out^T = h^T + (wp^T O^T) * (1/denom) + bp      (f32 combine)
Matmul operands are bf16 (1 cyc/row on PE); accumulation always fp32 in PSUM.
Host side transposes x per batch, casts weights to bf16, transposes output back.
"""

import numpy as np

B, L, C = 8, 2048, 512
GROUPS = 8
EPS = 1e-3
P = 128
CS = C // P            # 4 channel subtiles of 128
LQ = 512               # lq tile width (matmul free dim)
NLT = L // LQ          # 4 lq tiles
NLB = L // P           # 16 key/l blocks
CPG = C // GROUPS      # 64 channels per group
N_CORES = 8

_CACHE = {}


def _build_nc():
    from contextlib import ExitStack

    import concourse.bass as bass
    import concourse.mybir as mybir
    import concourse.tile as tile
    from concourse import bacc
    from concourse.bass import ts

    f32 = mybir.dt.float32
    bf16 = mybir.dt.bfloat16
    AF = mybir.ActivationFunctionType
    ALU = mybir.AluOpType

    nc = bacc.Bacc(trn_type="TRN2")

    xt_d = nc.dram_tensor("xt", [C, L], f32, kind="ExternalInput")
    w_d = {
        n: nc.dram_tensor(n, [P, CS, C], bf16, kind="ExternalInput")
        for n in ("wq", "wk", "wv", "wp")
    }
    vec_d = {
        n: nc.dram_tensor(n, [P, CS], f32, kind="ExternalInput")
        for n in ("gamma", "beta", "bq", "bk", "bp")
    }
    bvb_d = nc.dram_tensor("bv_bcast", [P, C], f32, kind="ExternalInput")
    g0_d = nc.dram_tensor("g0", [P, 2], f32, kind="ExternalInput")
    sel_d = nc.dram_tensor("sel", [2, P], f32, kind="ExternalInput")
    out_d = nc.dram_tensor("out_t", [C, L], f32, kind="ExternalOutput")

    xt_dv = xt_d[:].rearrange("(s p) l -> p s l", p=P)
    out_dv = out_d[:].rearrange("(s p) l -> p s l", p=P)

    with tile.TileContext(nc) as tc, ExitStack() as ctx:
        consts = ctx.enter_context(tc.tile_pool(name="consts", bufs=1))
        data = ctx.enter_context(tc.tile_pool(name="data", bufs=1))
        small = ctx.enter_context(tc.tile_pool(name="small", bufs=1))
        ptp = ctx.enter_context(tc.tile_pool(name="ptp", bufs=3))
        oup = ctx.enter_context(tc.tile_pool(name="oup", bufs=4))
        finp = ctx.enter_context(tc.tile_pool(name="finp", bufs=2))
        psA = ctx.enter_context(tc.tile_pool(name="psA", bufs=4, space="PSUM"))
        psS = ctx.enter_context(tc.tile_pool(name="psS", bufs=3, space="PSUM"))
        psD = ctx.enter_context(tc.tile_pool(name="psD", bufs=1, space="PSUM"))

        # ---- SBUF residents ----
        xt = data.tile([P, CS, L], f32)       # x^T, then h^T (f32, residual)
        hb = data.tile([P, CS, L], bf16)      # h^T bf16 (matmul operand)
        qt = data.tile([P, CS, L], bf16)      # Q^T
        kt = data.tile([P, CS, L], bf16)      # K^T (pre-scaled)
        vt = data.tile([P, NLB, C], bf16)     # V natural, [l%P, l//P, c]
        wsb = {n: consts.tile([P, CS, C], bf16, name=f"w_{n}") for n in w_d}
        vsb = {n: consts.tile([P, CS], f32, name=f"v_{n}") for n in vec_d}
        bvb = consts.tile([P, C], f32)
        g0 = consts.tile([P, 2], f32)
        sel = consts.tile([2, P], f32)
        ones_col = consts.tile([P, 1], bf16)
        ones_row = consts.tile([1, P], bf16)
        eps2 = consts.tile([2, 1], f32)

        # ---- loads + constants ----
        for s in range(CS):
            for j in range(4):
                nc.sync.dma_start(out=xt[:, s, ts(j, 512)], in_=xt_dv[:, s, ts(j, 512)])
        for n in w_d:
            nc.sync.dma_start(out=wsb[n][:], in_=w_d[n][:])
        for n in vec_d:
            nc.sync.dma_start(out=vsb[n][:], in_=vec_d[n][:])
        nc.sync.dma_start(out=bvb[:], in_=bvb_d[:])
        nc.sync.dma_start(out=g0[:], in_=g0_d[:])
        nc.sync.dma_start(out=sel[:], in_=sel_d[:])
        nc.vector.memset(ones_col[:], 1.0)
        nc.vector.memset(ones_row[:], 1.0)
        nc.vector.memset(eps2[:], EPS)

        # ---- GroupNorm stats ----
        # per-channel (partition) mean/var over L via bn_stats, then group
        # aggregation across partitions with a tiny fp32 matmul.
        st = small.tile([P, CS, 2], f32)      # (mean_c, E[x^2]_c) per subtile
        for s in range(CS):
            st6 = small.tile([P, 4, 6], f32, tag="st6", bufs=2)
            for j in range(4):
                nc.vector.bn_stats(out=st6[:, j, :], in_=xt[:, s, ts(j, 512)])
            mv = small.tile([P, 2], f32, tag="mv", bufs=2)
            nc.vector.bn_aggr(out=mv[:], in_=st6[:])
            nc.vector.tensor_copy(out=st[:, s, 0:1], in_=mv[:, 0:1])
            nc.vector.tensor_tensor(out=st[:, s, 1:2], in0=mv[:, 0:1], in1=mv[:, 0:1], op=ALU.mult)
            nc.vector.tensor_tensor(out=st[:, s, 1:2], in0=st[:, s, 1:2], in1=mv[:, 1:2], op=ALU.add)

        psg = psD.tile([2, 2 * CS], f32, tag="d")   # [group-half, (s, stat)]
        nc.tensor.matmul(psg[:], lhsT=g0[:], rhs=st[:].rearrange("p a b -> p (a b)"),
                         start=True, stop=True)
        pst = small.tile([2, 2 * CS], f32)
        nc.vector.tensor_copy(out=pst[:], in_=psg[:])
        pstv = pst[:].rearrange("p (s k) -> p s k", k=2)
        msq = small.tile([2, CS], f32)
        nc.vector.tensor_tensor(out=msq[:], in0=pstv[:, :, 0], in1=pstv[:, :, 0], op=ALU.mult)
        grp = small.tile([2, 2 * CS], f32)     # [:, :CS]=rstd_g, [:, CS:]=mean_g
        nc.vector.tensor_tensor(out=grp[:, 0:CS], in0=pstv[:, :, 1], in1=msq[:], op=ALU.subtract)
        nc.scalar.activation(out=grp[:, 0:CS], in_=grp[:, 0:CS], func=AF.Sqrt,
                             bias=eps2[:], scale=1.0)
        nc.vector.reciprocal(out=grp[:, 0:CS], in_=grp[:, 0:CS])
        nc.vector.tensor_copy(out=grp[:, CS:], in_=pstv[:, :, 0])

        psbc = psD.tile([P, 2 * CS], f32, tag="d")  # broadcast groups -> channels
        nc.tensor.matmul(psbc[:], lhsT=sel[:], rhs=grp[:], start=True, stop=True)
        ab = small.tile([P, 2 * CS], f32)      # [:, :CS]=a_c, [:, CS:]=b_c
        nc.vector.tensor_tensor(out=ab[:, 0:CS], in0=vsb["gamma"][:], in1=psbc[:, 0:CS], op=ALU.mult)
        nc.vector.tensor_tensor(out=ab[:, CS:], in0=psbc[:, CS:], in1=ab[:, 0:CS], op=ALU.mult)
        nc.vector.tensor_tensor(out=ab[:, CS:], in0=vsb["beta"][:], in1=ab[:, CS:], op=ALU.subtract)

        # ---- normalize: h^T = a*x^T + b ----
        # bf16 copy first (it gates all matmuls); the f32 in-place pass only
        # feeds the residual add much later, so it runs off the critical path.
        for s in range(CS):
            nc.vector.tensor_scalar(out=hb[:, s, :], in0=xt[:, s, :],
                                    scalar1=ab[:, s:s + 1], scalar2=ab[:, CS + s:CS + s + 1],
                                    op0=ALU.mult, op1=ALU.add)
        for s in range(CS):
            nc.vector.tensor_scalar(out=xt[:, s, :], in0=xt[:, s, :],
                                    scalar1=ab[:, s:s + 1], scalar2=ab[:, CS + s:CS + s + 1],
                                    op0=ALU.mult, op1=ALU.add)

        # ---- projections ----
        def project_t(w, bias, dst):
            # dst[:, co_s, l] = sum_ci w[ci, co]^T h^T + bias[co]
            for co_s in range(CS):
                for lt in range(NLT):
                    ps = psS.tile([P, LQ], f32, tag="s", name="ps_prj")
                    for ci in range(CS):
                        nc.tensor.matmul(ps[:], lhsT=w[:, ci, ts(co_s, P)],
                                         rhs=hb[:, ci, ts(lt, LQ)],
                                         start=(ci == 0), stop=(ci == CS - 1))
                    nc.scalar.activation(out=dst[:, co_s, ts(lt, LQ)], in_=ps[:],
                                         func=AF.Identity, bias=bias[:, co_s:co_s + 1], scale=1.0)

        project_t(wsb["wq"], vsb["bq"], qt)
        project_t(wsb["wk"], vsb["bk"], kt)

        for lb in range(NLB):
            ps = psS.tile([P, C], f32, tag="s", name="ps_v")
            for ci in range(CS):
                nc.tensor.matmul(ps[:], lhsT=hb[:, ci, ts(lb, P)],
                                 rhs=wsb["wv"][:, ci, :],
                                 start=(ci == 0), stop=(ci == CS - 1))
            nc.vector.tensor_add(out=vt[:, lb, :], in0=ps[:], in1=bvb[:])

        # ---- attention + output projection, per lq tile ----
        for lt in range(NLT):
            po = [psA.tile([P, LQ], f32, tag="po", name=f"po{i}") for i in range(CS)]
            pd = psD.tile([1, LQ], f32, tag="d", name="pd")
            for kb in range(NLB):
                ps = psS.tile([P, LQ], f32, tag="s", name="ps_s")
                for ci in range(CS):
                    nc.tensor.matmul(ps[:], lhsT=kt[:, ci, ts(kb, P)],
                                     rhs=qt[:, ci, ts(lt, LQ)],
                                     start=(ci == 0), stop=(ci == CS - 1))
                pt = ptp.tile([P, LQ], bf16, tag="pt")
                nc.scalar.activation(out=pt[:], in_=ps[:], func=AF.Exp)
                for c_ in range(CS):
                    nc.tensor.matmul(po[c_][:], lhsT=vt[:, kb, ts(c_, P)], rhs=pt[:],
                                     start=(kb == 0), stop=(kb == NLB - 1))
                nc.tensor.matmul(pd[:], lhsT=ones_col[:], rhs=pt[:],
                                 start=(kb == 0), stop=(kb == NLB - 1))

            # broadcast raw denominators across partitions via PE, then take the
            # reciprocal on all 128 lanes (a [1,512] single-lane reciprocal is
            # ~2.7us and stalls the PE; this way is ~0.4us off the PE path).
            pdc = small.tile([1, LQ], bf16, tag="pdc", bufs=2)
            with nc.allow_low_precision(reason="denom rounded to bf16 as matmul operand"):
                nc.vector.tensor_copy(out=pdc[:], in_=pd[:])
            pb = psS.tile([P, LQ], f32, tag="s", name="ps_b")
            nc.tensor.matmul(pb[:], lhsT=ones_row[:], rhs=pdc[:], start=True, stop=True)
            rb = finp.tile([P, LQ], f32, tag="rb")
            nc.vector.reciprocal(out=rb[:], in_=pb[:])

            ou = [oup.tile([P, LQ], bf16, tag="ou", name=f"ou{i}") for i in range(CS)]
            for c_ in range(CS):
                nc.vector.tensor_copy(out=ou[c_][:], in_=po[c_][:])

            for co_s in range(CS):
                pz = psS.tile([P, LQ], f32, tag="s", name="ps_z")
                for ci in range(CS):
                    nc.tensor.matmul(pz[:], lhsT=wsb["wp"][:, ci, ts(co_s, P)],
                                     rhs=ou[ci][:],
                                     start=(ci == 0), stop=(ci == CS - 1))
                fin = finp.tile([P, LQ], f32, tag="fin")
                nc.vector.tensor_tensor(out=fin[:], in0=pz[:], in1=rb[:], op=ALU.mult)
                nc.vector.tensor_scalar(out=fin[:], in0=fin[:],
                                        scalar1=vsb["bp"][:, co_s:co_s + 1], scalar2=None,
                                        op0=ALU.add)
                nc.vector.tensor_tensor(out=fin[:], in0=fin[:],
                                        in1=xt[:, co_s, ts(lt, LQ)], op=ALU.add)
                nc.sync.dma_start(out=out_dv[:, co_s, ts(lt, LQ)], in_=fin[:])

    nc.compile()
    return nc


def get_nc():
    if "nc" not in _CACHE:
        _CACHE["nc"] = _build_nc()
    return _CACHE["nc"]


def _g0_const():
    g = np.zeros((P, 2), np.float32)
    g[0:CPG, 0] = 1.0 / CPG
    g[CPG:P, 1] = 1.0 / CPG
    return g


def _sel_const():
    s = np.zeros((2, P), np.float32)
    s[0, 0:CPG] = 1.0
    s[1, CPG:P] = 1.0
    return s


def prep_inputs(x, gamma, beta, wq, bq, wk, bk, wv, bv, wp, bp):
    """Host-side layout prep (transposes / reshapes / bf16 weight casts, plus
    folding the 1/sqrt(C) attention scale into wk/bk). Per-core input maps."""
    import ml_dtypes

    f = np.float32
    bf = ml_dtypes.bfloat16
    x = np.asarray(x, f)
    scale = f(C) ** f(-0.5)

    def wprep(w):
        w = np.asarray(w, f)
        return np.ascontiguousarray(w.reshape(CS, P, C).transpose(1, 0, 2)).astype(bf)

    def vprep(v):
        v = np.asarray(v, f)
        return np.ascontiguousarray(v.reshape(CS, P).T)

    shared = {
        "wq": wprep(wq), "wk": wprep(np.asarray(wk, f) * scale),
        "wv": wprep(wv), "wp": wprep(wp),
        "gamma": vprep(gamma), "beta": vprep(beta),
        "bq": vprep(bq), "bk": vprep(np.asarray(bk, f) * scale), "bp": vprep(bp),
        "bv_bcast": np.ascontiguousarray(np.broadcast_to(np.asarray(bv, f), (P, C))),
        "g0": _g0_const(), "sel": _sel_const(),
    }
    in_maps = []
    for b in range(N_CORES):
        m = dict(shared)
        m["xt"] = np.ascontiguousarray(x[b].T)
        in_maps.append(m)
    return in_maps


def run(inputs, trace=False, **kw):
    from concourse.bass_utils import run_bass_kernel_spmd

    nc = get_nc()
    in_maps = prep_inputs(**inputs)
    return run_bass_kernel_spmd(nc, in_maps, core_ids=list(range(N_CORES)),
                                trace=trace, **kw)


def kernel(**inputs) -> np.ndarray:
    res = run(inputs)
    out = np.empty((B, L, C), np.float32)
    for b in range(N_CORES):
        out[b] = res.results[b]["out_t"].T
    return out


# revision 15
# speedup vs baseline: 1.1106x; 1.0559x over previous
"""Trainium2 Bass kernel for nn_AttentionBlock (B=8, L=2048, C=512, GroupNorm(8) +
single-head attention + residual), data-parallel over batch across 8 NeuronCores.

Self-contained: hardcodes shapes/sharding. kernel(**inputs) -> np.ndarray [B,L,C].

Dataflow (per core, one batch element, everything channel-major / "transposed"):
  x^T [C,L] (f32) --bn_stats/group-reduce--> h^T = a_c * x^T + b_c  (f32 + bf16 copy)
  Q^T = wq^T h^T + bq ;  K^T = (wk*scale)^T h^T + bk*scale  (scale folded on host)
  V   = h^T-chunks^T @ wv + bv           (natural [L,C] layout)
  per 512-wide lq tile:
     for each 128-key block: S^T = K^T-chunk^T @ Q^T (PSUM f32); P = exp(S^T) (bf16)
     O^T  += V-chunk^T @ P  (PSUM f32 accum over key blocks), denom += 1^T @ P
     out^T = h^T + (wp^T O^T) * (1/denom) + bp      (f32 combine)
Matmul operands are bf16 (1 cyc/row on PE); accumulation always fp32 in PSUM.
Host side transposes x per batch, casts weights to bf16, transposes output back.
"""

import numpy as np

B, L, C = 8, 2048, 512
GROUPS = 8
EPS = 1e-3
P = 128
CS = C // P            # 4 channel subtiles of 128
LQ = 512               # lq tile width (matmul free dim)
NLT = L // LQ          # 4 lq tiles
NLB = L // P           # 16 key/l blocks
CPG = C // GROUPS      # 64 channels per group
N_CORES = 8

_CACHE = {}


def _build_nc():
    from contextlib import ExitStack

    import concourse.bass as bass
    import concourse.mybir as mybir
    import concourse.tile as tile
    from concourse import bacc
    from concourse.bass import ts

    f32 = mybir.dt.float32
    bf16 = mybir.dt.bfloat16
    AF = mybir.ActivationFunctionType
    ALU = mybir.AluOpType

    nc = bacc.Bacc(trn_type="TRN2")

    xt_d = nc.dram_tensor("xt", [C, L], f32, kind="ExternalInput")
    w_d = {
        n: nc.dram_tensor(n, [P, CS, C], bf16, kind="ExternalInput")
        for n in ("wq", "wk", "wv", "wp")
    }
    vec_d = {
        n: nc.dram_tensor(n, [P, CS], f32, kind="ExternalInput")
        for n in ("gamma", "beta", "bq", "bk", "bp")
    }
    bvb_d = nc.dram_tensor("bv_bcast", [P, C], f32, kind="ExternalInput")
    g0_d = nc.dram_tensor("g0", [P, 2], f32, kind="ExternalInput")
    sel_d = nc.dram_tensor("sel", [2, P], f32, kind="ExternalInput")
    out_d = nc.dram_tensor("out_t", [C, L], f32, kind="ExternalOutput")

    xt_dv = xt_d[:].rearrange("(s p) l -> p s l", p=P)
    out_dv = out_d[:].rearrange("(s p) l -> p s l", p=P)

    with tile.TileContext(nc) as tc, ExitStack() as ctx:
        consts = ctx.enter_context(tc.tile_pool(name="consts", bufs=1))
        data = ctx.enter_context(tc.tile_pool(name="data", bufs=1))
        small = ctx.enter_context(tc.tile_pool(name="small", bufs=1))
        ptp = ctx.enter_context(tc.tile_pool(name="ptp", bufs=3))
        oup = ctx.enter_context(tc.tile_pool(name="oup", bufs=4))
        finp = ctx.enter_context(tc.tile_pool(name="finp", bufs=2))
        psA = ctx.enter_context(tc.tile_pool(name="psA", bufs=4, space="PSUM"))
        psS = ctx.enter_context(tc.tile_pool(name="psS", bufs=3, space="PSUM"))
        psD = ctx.enter_context(tc.tile_pool(name="psD", bufs=1, space="PSUM"))

        # ---- SBUF residents ----
        xt = data.tile([P, CS, L], f32)       # x^T, then h^T (f32, residual)
        hb = data.tile([P, CS, L], bf16)      # h^T bf16 (matmul operand)
        qt = data.tile([P, CS, L], bf16)      # Q^T
        kt = data.tile([P, CS, L], bf16)      # K^T (pre-scaled)
        vt = data.tile([P, NLB, C], bf16)     # V natural, [l%P, l//P, c]
        wsb = {n: consts.tile([P, CS, C], bf16, name=f"w_{n}") for n in w_d}
        vsb = {n: consts.tile([P, CS], f32, name=f"v_{n}") for n in vec_d}
        bvb = consts.tile([P, C], f32)
        g0 = consts.tile([P, 2], f32)
        sel = consts.tile([2, P], f32)
        ones_col = consts.tile([P, 1], bf16)
        ones_row = consts.tile([1, P], bf16)
        eps2 = consts.tile([2, 1], f32)

        # ---- loads + constants ----
        # 2 DMAs per channel-subtile: few enough that the Sync engine's ~650ns
        # per-DMA issue cost doesn't serialize the load, fine enough that
        # bn_stats can start before the whole 4MB lands.
        for s in range(CS):
            for j in range(2):
                nc.sync.dma_start(out=xt[:, s, ts(j, 1024)], in_=xt_dv[:, s, ts(j, 1024)])
        for n in w_d:
            nc.sync.dma_start(out=wsb[n][:], in_=w_d[n][:])
        for n in vec_d:
            nc.sync.dma_start(out=vsb[n][:], in_=vec_d[n][:])
        nc.sync.dma_start(out=bvb[:], in_=bvb_d[:])
        nc.sync.dma_start(out=g0[:], in_=g0_d[:])
        nc.sync.dma_start(out=sel[:], in_=sel_d[:])
        nc.vector.memset(ones_col[:], 1.0)
        nc.vector.memset(ones_row[:], 1.0)
        nc.vector.memset(eps2[:], EPS)

        # ---- GroupNorm stats ----
        # per-channel (partition) mean/var over L via bn_stats, then group
        # aggregation across partitions with a tiny fp32 matmul.
        st = small.tile([P, CS, 2], f32)      # (mean_c, E[x^2]_c) per subtile
        for s in range(CS):
            st6 = small.tile([P, 4, 6], f32, tag="st6", bufs=2)
            for j in range(4):
                nc.vector.bn_stats(out=st6[:, j, :], in_=xt[:, s, ts(j, 512)])
            mv = small.tile([P, 2], f32, tag="mv", bufs=2)
            nc.vector.bn_aggr(out=mv[:], in_=st6[:])
            nc.vector.tensor_copy(out=st[:, s, 0:1], in_=mv[:, 0:1])
            nc.vector.tensor_tensor(out=st[:, s, 1:2], in0=mv[:, 0:1], in1=mv[:, 0:1], op=ALU.mult)
            nc.vector.tensor_tensor(out=st[:, s, 1:2], in0=st[:, s, 1:2], in1=mv[:, 1:2], op=ALU.add)

        psg = psD.tile([2, 2 * CS], f32, tag="d")   # [group-half, (s, stat)]
        nc.tensor.matmul(psg[:], lhsT=g0[:], rhs=st[:].rearrange("p a b -> p (a b)"),
                         start=True, stop=True)
        pst = small.tile([2, 2 * CS], f32)
        nc.vector.tensor_copy(out=pst[:], in_=psg[:])
        pstv = pst[:].rearrange("p (s k) -> p s k", k=2)
        msq = small.tile([2, CS], f32)
        nc.vector.tensor_tensor(out=msq[:], in0=pstv[:, :, 0], in1=pstv[:, :, 0], op=ALU.mult)
        grp = small.tile([2, 2 * CS], f32)     # [:, :CS]=rstd_g, [:, CS:]=mean_g
        nc.vector.tensor_tensor(out=grp[:, 0:CS], in0=pstv[:, :, 1], in1=msq[:], op=ALU.subtract)
        nc.scalar.activation(out=grp[:, 0:CS], in_=grp[:, 0:CS], func=AF.Sqrt,
                             bias=eps2[:], scale=1.0)
        nc.vector.reciprocal(out=grp[:, 0:CS], in_=grp[:, 0:CS])
        nc.vector.tensor_copy(out=grp[:, CS:], in_=pstv[:, :, 0])

        psbc = psD.tile([P, 2 * CS], f32, tag="d")  # broadcast groups -> channels
        nc.tensor.matmul(psbc[:], lhsT=sel[:], rhs=grp[:], start=True, stop=True)
        ab = small.tile([P, 2 * CS], f32)      # [:, :CS]=a_c, [:, CS:]=b_c
        nc.vector.tensor_tensor(out=ab[:, 0:CS], in0=vsb["gamma"][:], in1=psbc[:, 0:CS], op=ALU.mult)
        nc.vector.tensor_tensor(out=ab[:, CS:], in0=psbc[:, CS:], in1=ab[:, 0:CS], op=ALU.mult)
        nc.vector.tensor_tensor(out=ab[:, CS:], in0=vsb["beta"][:], in1=ab[:, CS:], op=ALU.subtract)

        # ---- normalize: h^T = a*x^T + b ----
        # bf16 copy first (it gates all matmuls), split across DVE and ACT so
        # all four subtiles are ready ~2x sooner; the f32 in-place pass only
        # feeds the residual add much later, so it runs off the critical path.
        for s in range(CS):
            if s < 2:
                nc.vector.tensor_scalar(out=hb[:, s, :], in0=xt[:, s, :],
                                        scalar1=ab[:, s:s + 1], scalar2=ab[:, CS + s:CS + s + 1],
                                        op0=ALU.mult, op1=ALU.add)
            else:
                nc.scalar.activation(out=hb[:, s, :], in_=xt[:, s, :], func=AF.Identity,
                                     bias=ab[:, CS + s:CS + s + 1], scale=ab[:, s:s + 1])
        for s in range(CS):
            nc.vector.tensor_scalar(out=xt[:, s, :], in0=xt[:, s, :],
                                    scalar1=ab[:, s:s + 1], scalar2=ab[:, CS + s:CS + s + 1],
                                    op0=ALU.mult, op1=ALU.add)

        # ---- projections ----
        def project_t(w, bias, dst):
            # dst[:, co_s, l] = sum_ci w[ci, co]^T h^T + bias[co]
            for co_s in range(CS):
                for lt in range(NLT):
                    ps = psS.tile([P, LQ], f32, tag="s", name="ps_prj")
                    for ci in range(CS):
                        nc.tensor.matmul(ps[:], lhsT=w[:, ci, ts(co_s, P)],
                                         rhs=hb[:, ci, ts(lt, LQ)],
                                         start=(ci == 0), stop=(ci == CS - 1))
                    nc.scalar.activation(out=dst[:, co_s, ts(lt, LQ)], in_=ps[:],
                                         func=AF.Identity, bias=bias[:, co_s:co_s + 1], scale=1.0)

        project_t(wsb["wq"], vsb["bq"], qt)
        project_t(wsb["wk"], vsb["bk"], kt)

        for lb in range(NLB):
            ps = psS.tile([P, C], f32, tag="s", name="ps_v")
            for ci in range(CS):
                nc.tensor.matmul(ps[:], lhsT=hb[:, ci, ts(lb, P)],
                                 rhs=wsb["wv"][:, ci, :],
                                 start=(ci == 0), stop=(ci == CS - 1))
            nc.vector.tensor_add(out=vt[:, lb, :], in0=ps[:], in1=bvb[:])

        # ---- attention + output projection, per lq tile ----
        for lt in range(NLT):
            po = [psA.tile([P, LQ], f32, tag="po", name=f"po{i}") for i in range(CS)]
            pd = psD.tile([1, LQ], f32, tag="d", name="pd")
            for kb in range(NLB):
                ps = psS.tile([P, LQ], f32, tag="s", name="ps_s")
                for ci in range(CS):
                    nc.tensor.matmul(ps[:], lhsT=kt[:, ci, ts(kb, P)],
                                     rhs=qt[:, ci, ts(lt, LQ)],
                                     start=(ci == 0), stop=(ci == CS - 1))
                pt = ptp.tile([P, LQ], bf16, tag="pt")
                nc.scalar.activation(out=pt[:], in_=ps[:], func=AF.Exp)
                for c_ in range(CS):
                    nc.tensor.matmul(po[c_][:], lhsT=vt[:, kb, ts(c_, P)], rhs=pt[:],
                                     start=(kb == 0), stop=(kb == NLB - 1))
                nc.tensor.matmul(pd[:], lhsT=ones_col[:], rhs=pt[:],
                                 start=(kb == 0), stop=(kb == NLB - 1))

            # Finale. Order matters: pdc frees the "d" bank and the ou copies
            # free the "po" banks that the next lq tile's denominator/PV
            # matmuls need — emit them first so DVE runs them first.
            # Broadcast raw denominators across partitions via PE, then take
            # the reciprocal on all 128 lanes (a [1,512] single-lane
            # reciprocal is ~2.7us and stalls the PE).
            pdc = small.tile([1, LQ], bf16, tag="pdc", bufs=2)
            with nc.allow_low_precision(reason="denom rounded to bf16 as matmul operand"):
                nc.vector.tensor_copy(out=pdc[:], in_=pd[:])
            ou = [oup.tile([P, LQ], bf16, tag="ou", name=f"ou{i}") for i in range(CS)]
            for c_ in range(CS):
                nc.vector.tensor_copy(out=ou[c_][:], in_=po[c_][:])
            pb = psA.tile([P, LQ], f32, tag="po", name="ps_b")
            nc.tensor.matmul(pb[:], lhsT=ones_row[:], rhs=pdc[:], start=True, stop=True)
            rb = finp.tile([P, LQ], f32, tag="rb")
            nc.vector.reciprocal(out=rb[:], in_=pb[:])

            for co_s in range(CS):
                pz = psA.tile([P, LQ], f32, tag="po", name="ps_z")
                for ci in range(CS):
                    nc.tensor.matmul(pz[:], lhsT=wsb["wp"][:, ci, ts(co_s, P)],
                                     rhs=ou[ci][:],
                                     start=(ci == 0), stop=(ci == CS - 1))
                fin = finp.tile([P, LQ], f32, tag="fin")
                nc.vector.tensor_tensor(out=fin[:], in0=pz[:], in1=rb[:], op=ALU.mult)
                nc.vector.tensor_scalar(out=fin[:], in0=fin[:],
                                        scalar1=vsb["bp"][:, co_s:co_s + 1], scalar2=None,
                                        op0=ALU.add)
                nc.vector.tensor_tensor(out=fin[:], in0=fin[:],
                                        in1=xt[:, co_s, ts(lt, LQ)], op=ALU.add)
                nc.sync.dma_start(out=out_dv[:, co_s, ts(lt, LQ)], in_=fin[:])

    nc.compile()
    return nc


def get_nc():
    if "nc" not in _CACHE:
        _CACHE["nc"] = _build_nc()
    return _CACHE["nc"]


def _g0_const():
    g = np.zeros((P, 2), np.float32)
    g[0:CPG, 0] = 1.0 / CPG
    g[CPG:P, 1] = 1.0 / CPG
    return g


def _sel_const():
    s = np.zeros((2, P), np.float32)
    s[0, 0:CPG] = 1.0
    s[1, CPG:P] = 1.0
    return s


def prep_inputs(x, gamma, beta, wq, bq, wk, bk, wv, bv, wp, bp):
    """Host-side layout prep (transposes / reshapes / bf16 weight casts, plus
    folding the 1/sqrt(C) attention scale into wk/bk). Per-core input maps."""
    import ml_dtypes

    f = np.float32
    bf = ml_dtypes.bfloat16
    x = np.asarray(x, f)
    scale = f(C) ** f(-0.5)

    def wprep(w):
        w = np.asarray(w, f)
        return np.ascontiguousarray(w.reshape(CS, P, C).transpose(1, 0, 2)).astype(bf)

    def vprep(v):
        v = np.asarray(v, f)
        return np.ascontiguousarray(v.reshape(CS, P).T)

    shared = {
        "wq": wprep(wq), "wk": wprep(np.asarray(wk, f) * scale),
        "wv": wprep(wv), "wp": wprep(wp),
        "gamma": vprep(gamma), "beta": vprep(beta),
        "bq": vprep(bq), "bk": vprep(np.asarray(bk, f) * scale), "bp": vprep(bp),
        "bv_bcast": np.ascontiguousarray(np.broadcast_to(np.asarray(bv, f), (P, C))),
        "g0": _g0_const(), "sel": _sel_const(),
    }
    in_maps = []
    for b in range(N_CORES):
        m = dict(shared)
        m["xt"] = np.ascontiguousarray(x[b].T)
        in_maps.append(m)
    return in_maps


def run(inputs, trace=False, **kw):
    from concourse.bass_utils import run_bass_kernel_spmd

    nc = get_nc()
    in_maps = prep_inputs(**inputs)
    return run_bass_kernel_spmd(nc, in_maps, core_ids=list(range(N_CORES)),
                                trace=trace, **kw)


def kernel(**inputs) -> np.ndarray:
    res = run(inputs)
    out = np.empty((B, L, C), np.float32)
    for b in range(N_CORES):
        out[b] = res.results[b]["out_t"].T
    return out
